# revision 1
# baseline (speedup 1.0000x reference)
"""CIoU kernel v3 (T=64): as kernel v2 but with halved instruction count and
SBUF aliasing so the larger chunk fits:
  - 9x9 padded C (wrap row/col appended once, core at [0:8,0:8]);
  - 9-slot SA/SB pads; rAc aliases cpad, qA/qB aliases w1p (slice views);
  - fp16 bridge tiles carved out of dead sap/sbp regions via bitcast;
  - shf kept fp16 (+-64 exact), mixed-dtype Pool subtract (probed legal).
Algorithm identical to kernel v2 (validated: rel_err 2.3e-05).
"""
import sys

sys.path.insert(0, "/opt/trn_rl_repo")

import numpy as np
import concourse.bass as bass
import concourse.bacc as bacc
import concourse.tile as tile
from concourse import mybir
from concourse.bass_utils import run_bass_kernel_spmd

AOT = mybir.AluOpType
ACT = mybir.ActivationFunctionType
F32 = mybir.dt.float32
F16 = mybir.dt.float16

B = 262144
NCORES = 8
NI = B // NCORES
P = 128
EPS = 1e-12
BIG = 1e20
SHIFT = 64.0
TINY = 1e-30


ASSIGN = {}


def build_program(T=64, assign=None):
    global ASSIGN
    ASSIGN = dict(assign or {})
    CH = P * T
    NCH = NI // CH
    nc = bacc.Bacc("TRN2", target_bir_lowering=False, debug=False, num_devices=NCORES)
    ab_d = nc.dram_tensor("ab", [NI, 32], F32, kind="ExternalInput")
    out_d = nc.dram_tensor("ciou", [NI], F32, kind="ExternalOutput")

    with tile.TileContext(nc) as tc:
        with tc.tile_pool(name="pool", bufs=1) as pool, \
             tc.tile_pool(name="spool", bufs=1) as spool:
            tb = spool.tile([P, 1], F32, tag="tinyb", name="tinyb")
            nc.gpsimd.memset(tb, TINY)
            for ch in range(NCH):
                _chunk(nc, pool, spool, ab_d, out_d, ch, T, tb)
    nc.compile()
    return nc


def _chunk(nc, pool, spool, ab_d, out_d, ch, T, tb):
    CH = P * T
    v = nc.vector
    g = nc.gpsimd
    s = nc.scalar

    def E(site, default):
        return {"v": v, "g": g, "s": s}[ASSIGN.get(site, default)]

    def big(tag, d=F32, n=64):
        return pool.tile([P, n * T], d, tag=tag, name=tag)

    def small(tag, d=F32, n=8):
        return spool.tile([P, n * T], d, tag=tag, name=tag)

    def tiny(tag, d=F32):
        return spool.tile([P, T], d, tag=tag, name=tag)

    def dn(tl):
        return tl.rearrange("p (i k t) -> p i k t", i=8, k=8)

    # ---------------- load ----------------
    raw = pool.tile([P, 32 * T], F32, tag="raw", name="raw")
    ab_view = ab_d[ch * CH:(ch + 1) * CH, :].rearrange("(p t) jc -> p (t jc)", p=P)
    nc.sync.dma_start(raw, ab_view)

    rr = raw.rearrange("p (t h j c) -> p h j c t", h=2, j=8, c=2)
    ax = rr[:, 0, :, 0, :]; ay = rr[:, 0, :, 1, :]
    bx = rr[:, 1, :, 0, :]; by = rr[:, 1, :, 1, :]

    # ---------------- adjacency (smalls) ----------------
    ua1 = small("adjt1"); ua2 = small("adjt2")
    va1 = ua1.rearrange("p (s t) -> p s t", s=8)
    va2 = ua2.rearrange("p (s t) -> p s t", s=8)
    v.tensor_tensor(va1[:, 0:7, :], ax[:, 0:7, :], ay[:, 1:8, :], AOT.mult)
    v.tensor_tensor(va2[:, 0:7, :], ay[:, 0:7, :], ax[:, 1:8, :], AOT.mult)
    v.tensor_tensor(va1[:, 7, :], ax[:, 7, :], ay[:, 0, :], AOT.mult)
    v.tensor_tensor(va2[:, 7, :], ay[:, 7, :], ax[:, 0, :], AOT.mult)
    adjAB = small("adjab", n=16)
    adjAB_v = adjAB.rearrange("p (s t) -> p s t", s=16)
    adjA_c = adjAB_v[:, 0:8, :]
    v.tensor_tensor(adjA_c, va1, va2, AOT.subtract)

    ub1 = small("adjt1"); ub2 = small("adjt2")
    vb1 = ub1.rearrange("p (s t) -> p s t", s=8)
    vb2 = ub2.rearrange("p (s t) -> p s t", s=8)
    g.tensor_tensor(vb1[:, 0:7, :], bx[:, 0:7, :], by[:, 1:8, :], AOT.mult)
    g.tensor_tensor(vb2[:, 0:7, :], by[:, 0:7, :], bx[:, 1:8, :], AOT.mult)
    g.tensor_tensor(vb1[:, 7, :], bx[:, 7, :], by[:, 0, :], AOT.mult)
    g.tensor_tensor(vb2[:, 7, :], by[:, 7, :], bx[:, 0, :], AOT.mult)
    adjB_c = adjAB_v[:, 8:16, :]
    g.tensor_tensor(adjB_c, vb1, vb2, AOT.subtract)

    adjA_ik = adjA_c.unsqueeze(2).broadcast_to((P, 8, 8, T))
    adjB_ik = adjB_c.unsqueeze(1).broadcast_to((P, 8, 8, T))

    # ---------------- 9x9 padded C (core [0:8,0:8], wrap row/col 8) --------
    t1 = big("t1")
    t2 = big("t2")
    ax_b = ax.unsqueeze(2).broadcast_to((P, 8, 8, T))
    ay_b = ay.unsqueeze(2).broadcast_to((P, 8, 8, T))
    bx_b = bx.unsqueeze(1).broadcast_to((P, 8, 8, T))
    by_b = by.unsqueeze(1).broadcast_to((P, 8, 8, T))
    v.tensor_tensor(dn(t1), ax_b, by_b, AOT.mult)
    g.tensor_tensor(dn(t2), ay_b, bx_b, AOT.mult)

    cpad = big("cpad", n=81)
    cp = cpad.rearrange("p (si sk t) -> p si sk t", si=9, sk=9)
    C0 = cp[:, 0:8, 0:8, :]
    v.tensor_tensor(C0, dn(t1), dn(t2), AOT.subtract)
    s.copy(cp[:, 0:8, 8, :], cp[:, 0:8, 0, :])      # col 8 <- col 0
    s.copy(cp[:, 8, :, :], cp[:, 0, :, :])          # row 8 <- row 0 (incl corner)

    # fp16 C for the bridge sum
    cf = big("cf", d=F16)
    s.copy(dn(cf), C0)

    # ---------------- SA / SB / D ----------------
    # w1[i, q] = C[i+1, q] - C[i, q], q = 0..8
    w1p = big("w1p", n=72)
    w1v = w1p.rearrange("p (i q t) -> p i q t", i=8, q=9)
    v.tensor_tensor(w1v, cp[:, 1:9, :, :], cp[:, 0:8, :, :], AOT.subtract)

    sap = big("sap", n=72)          # 9 i-slots: SAm=0:8, SA=1:9, slot0<-slot8
    sav = sap.rearrange("p (si k t) -> p si k t", si=9, k=8)
    SA = sav[:, 1:9, :, :]
    E("sa", "g").tensor_tensor(SA, w1v[:, :, 0:8, :], adjA_ik, AOT.add)
    s.copy(sav[:, 0, :, :], sav[:, 8, :, :])

    w2 = big("w2")
    E("w2", "g").tensor_tensor(dn(w2), cp[:, 0:8, 0:8, :], cp[:, 0:8, 1:9, :], AOT.subtract)
    sbp = big("sbp", n=72)          # 9 k-slots: SBm=0:8, SB=1:9, slot0<-slot8
    sbv = sbp.rearrange("p (i sk t) -> p i sk t", i=8, sk=9)
    SB = sbv[:, :, 1:9, :]
    E("sb", "v").tensor_tensor(SB, dn(w2), adjB_ik, AOT.add)
    s.copy(sbv[:, :, 0, :], sbv[:, :, 8, :])

    D = big("t1")                   # reuse t1 slot
    E("d", "g").tensor_tensor(dn(D), w1v[:, :, 1:9, :], w1v[:, :, 0:8, :], AOT.subtract)

    # ---------------- D sign chain ----------------
    sgnf = big("sgn", d=F16)
    s.activation(sgnf, D, ACT.Sign, bias=tb)
    Dsafe = big("t2")               # reuse t2 slot
    v.scalar_tensor_tensor(Dsafe, sgnf, EPS, D, op0=AOT.mult, op1=AOT.add)
    R = big("r")
    v.reciprocal(R, Dsafe)
    Rv = R.rearrange("p (i k t) -> p i k t", i=8, k=8)
    shf = big("shf", d=F16)
    g.tensor_scalar(shf, sgnf, SHIFT, None, AOT.mult)

    # ---------------- clip chains (f32) ----------------
    ratioA = big("t1")              # reuse
    v.tensor_tensor(dn(ratioA), SB, Rv, AOT.mult)
    rAc = cpad[:, 0:64 * T]         # alias: C table is dead by now
    g.tensor_scalar(rAc, ratioA, 4.0, -4.0, AOT.min, AOT.max)
    qA = w1p[:, 0:64 * T]           # alias: w1 dead after SA/D
    g.tensor_tensor(qA, rAc, shf, AOT.subtract)
    loA = small("loa")
    v.tensor_reduce(loA.rearrange("p (i t) -> p i t", i=8),
                    qA.rearrange("p (i k t) -> p i t k", i=8, k=8),
                    axis=mybir.AxisListType.X, op=AOT.max)
    hiA = small("hia")
    v.tensor_reduce(hiA.rearrange("p (i t) -> p i t", i=8),
                    qA.rearrange("p (i k t) -> p i t k", i=8, k=8),
                    axis=mybir.AxisListType.X, op=AOT.min)

    ratioB = big("t2")              # reuse
    E("rb", "v").tensor_tensor(dn(ratioB), SA, Rv, AOT.mult)
    rBc = cpad[:, 0:64 * T]         # alias (rAc consumed by qA)
    g.tensor_scalar(rBc, ratioB, 4.0, -4.0, AOT.min, AOT.max)
    qB = big("w2")                  # w2 is dead after SB; stays on Pool
    g.tensor_tensor(qB, rBc, shf, AOT.subtract)
    loB = sgnf[:, 0:16 * T].bitcast(F32)
    v.tensor_reduce(loB.rearrange("p (k t) -> p k t", k=8),
                    qB.rearrange("p (i k t) -> p k t i", i=8, k=8),
                    axis=mybir.AxisListType.X, op=AOT.min)
    hiB = sgnf[:, 16 * T:32 * T].bitcast(F32)
    v.tensor_reduce(hiB.rearrange("p (k t) -> p k t", k=8),
                    qB.rearrange("p (i k t) -> p k t i", i=8, k=8),
                    axis=mybir.AxisListType.X, op=AOT.max)

    # widths + weighted sums (smalls); shift constants folded in
    hiA2 = R[:, 0:8 * T]
    v.tensor_scalar(hiA2, hiA, SHIFT, 1.0, AOT.add, AOT.min)
    loA2 = R[:, 8 * T:16 * T]
    v.tensor_scalar(loA2, loA, -SHIFT, 0.0, AOT.add, AOT.max)
    wA = R[:, 16 * T:24 * T]
    v.tensor_tensor(wA, hiA2, loA2, AOT.subtract)
    wAr = R[:, 24 * T:32 * T]
    v.tensor_scalar_max(wAr, wA, 0.0)
    wadjAB = ratioA[:, 16 * T:32 * T]
    wadjAB_v = wadjAB.rearrange("p (s t) -> p s t", s=16)
    v.tensor_tensor(wadjAB_v[:, 0:8, :],
                    wAr.rearrange("p (i t) -> p i t", i=8), adjA_c, AOT.mult)

    loB2 = R[:, 32 * T:40 * T]
    v.tensor_scalar(loB2, loB, SHIFT, 0.0, AOT.add, AOT.min)
    hiB2 = R[:, 40 * T:48 * T]
    v.tensor_scalar(hiB2, hiB, -SHIFT, -1.0, AOT.add, AOT.max)
    wB = R[:, 48 * T:56 * T]
    v.tensor_tensor(wB, loB2, hiB2, AOT.subtract)
    wBr = R[:, 56 * T:64 * T]
    v.tensor_scalar_max(wBr, wB, 0.0)
    v.tensor_tensor(wadjAB_v[:, 8:16, :],
                    wBr.rearrange("p (k t) -> p k t", k=8), adjB_c, AOT.mult)
    isum = tiny("isum")
    v.tensor_reduce(isum, wadjAB.rearrange("p (s t) -> p t s", s=16),
                    axis=mybir.AxisListType.X, op=AOT.add)

    # ---------------- fp16 casts of SA/SB (Act) ----------------
    saf = big("saf", d=F16, n=72)
    safv = saf.rearrange("p (si k t) -> p si k t", si=9, k=8)
    s.copy(saf, sap)
    sbf = big("sbf", d=F16, n=72)
    sbfv = sbf.rearrange("p (i sk t) -> p i sk t", i=8, sk=9)
    s.copy(sbf, sbp)

    SAf = safv[:, 1:9, :, :]
    SAm = safv[:, 0:8, :, :]
    SBf = sbfv[:, :, 1:9, :]
    SBm = sbfv[:, :, 0:8, :]

    # ---------------- hull: surviving edges ----------------
    # pairwise fp16 min-tree over k (2x DVE tt beats no-speedup reduce; exact)
    mm1 = sap[:, 0:16 * T].bitcast(F16)     # sap consumers (ratioB/saf) done
    m1v = mm1.rearrange("p (i k t) -> p i k t", i=8, k=4)
    v.tensor_tensor(m1v, SAf[:, :, 0:4, :], SAf[:, :, 4:8, :], AOT.min)
    mm2 = w1p[:, 64 * T:72 * T].bitcast(F16)
    m2v = mm2.rearrange("p (i k t) -> p i k t", i=8, k=2)
    v.tensor_tensor(m2v, m1v[:, :, 0:2, :], m1v[:, :, 2:4, :], AOT.min)
    minSA = small("minsa", d=F16)
    v.tensor_tensor(minSA.rearrange("p (i t) -> p i t", i=8),
                    m2v[:, :, 0, :], m2v[:, :, 1, :], AOT.min)
    okEA = small("okea")
    v.tensor_scalar(okEA, minSA, 0.0, None, AOT.is_ge)
    eAB = ratioA[:, 0:16 * T]
    eAB_v = eAB.rearrange("p (s t) -> p s t", s=16)
    v.tensor_tensor(eAB_v[:, 0:8, :],
                    okEA.rearrange("p (i t) -> p i t", i=8), adjA_c, AOT.mult)
    mb1 = sbp[:, 0:16 * T].bitcast(F16)     # sbp consumers (ratioA/sbf) done
    mb1v = mb1.rearrange("p (i k t) -> p i k t", i=4, k=8)
    v.tensor_tensor(mb1v, SBf[:, 0:4, :, :], SBf[:, 4:8, :, :], AOT.min)
    mb2 = sap[:, 16 * T:24 * T].bitcast(F16)
    mb2v = mb2.rearrange("p (i k t) -> p i k t", i=2, k=8)
    v.tensor_tensor(mb2v, mb1v[:, 0:2, :, :], mb1v[:, 2:4, :, :], AOT.min)
    minSB = small("minsa", d=F16)   # reuse
    v.tensor_tensor(minSB.rearrange("p (k t) -> p k t", k=8),
                    mb2v[:, 0, :, :], mb2v[:, 1, :, :], AOT.min)
    okEB = small("okea")            # reuse
    v.tensor_scalar(okEB, minSB, 0.0, None, AOT.is_ge)
    v.tensor_tensor(eAB_v[:, 8:16, :],
                    okEB.rearrange("p (k t) -> p k t", k=8), adjB_c, AOT.mult)
    h1 = tiny("h1")
    v.tensor_reduce(h1, eAB.rearrange("p (s t) -> p t s", s=16),
                    axis=mybir.AxisListType.X, op=AOT.add)

    # ---------------- hull: bridges (fp16, carved from sap/sbp) ----------
    u = sap[:, 0:32 * T].bitcast(F16)
    v.tensor_tensor(dn(u), SAm, SBf, AOT.min)
    vv = sap[:, 32 * T:64 * T].bitcast(F16)
    v.tensor_tensor(dn(vv), SAf, SBm, AOT.max)
    s1 = sbp[:, 0:32 * T].bitcast(F16)
    v.scalar_tensor_tensor(s1, vv, -1.0, u, op0=AOT.mult, op1=AOT.min)
    mAB = sbp[:, 32 * T:64 * T].bitcast(F16)
    v.tensor_scalar(mAB, s1, 0.0, None, AOT.is_ge)
    u2 = sap[:, 0:32 * T].bitcast(F16)
    v.tensor_tensor(dn(u2), SAm, SBf, AOT.max)
    v2 = sap[:, 32 * T:64 * T].bitcast(F16)
    v.tensor_tensor(dn(v2), SAf, SBm, AOT.min)
    s2 = sbp[:, 0:32 * T].bitcast(F16)
    v.scalar_tensor_tensor(s2, u2, -1.0, v2, op0=AOT.mult, op1=AOT.min)
    mBA = sap[:, 0:32 * T].bitcast(F16)
    v.tensor_scalar(mBA, s2, 0.0, None, AOT.is_ge)
    mB = sap[:, 32 * T:64 * T].bitcast(F16)
    v.tensor_tensor(mB, mAB, mBA, AOT.subtract)
    cM = sbp[:, 0:32 * T].bitcast(F16)
    v.tensor_tensor(cM, mB, cf, AOT.mult)
    redBR = tiny("redbr")
    v.tensor_reduce(redBR, cM.rearrange("p (i k t) -> p t i k", i=8, k=8),
                    axis=mybir.AxisListType.XY, op=AOT.add)

    # ---------------- per-item finals ----------------
    asum = tiny("asum")
    v.tensor_reduce(asum, adjAB.rearrange("p (s t) -> p t s", s=16),
                    axis=mybir.AxisListType.X, op=AOT.add)
    inter = tiny("inter")
    s.mul(inter, isum, 0.5)
    union = tiny("union")
    v.scalar_tensor_tensor(union, asum, 0.5, inter, op0=AOT.mult, op1=AOT.subtract)
    hsum = tiny("hsum")
    v.tensor_tensor(hsum, h1, redBR, AOT.add)
    rcu = tiny("rcu")
    v.reciprocal(rcu, union)
    rch = tiny("rch")
    v.reciprocal(rch, hsum)
    iou = tiny("iou")
    v.tensor_tensor(iou, inter, rcu, AOT.mult)
    ioum1 = tiny("ioum1")
    v.tensor_scalar_add(ioum1, iou, -1.0)
    qq = tiny("qq")
    v.tensor_tensor(qq, union, rch, AOT.mult)
    ciou = tiny("ciou")
    v.scalar_tensor_tensor(ciou, qq, 2.0, ioum1, op0=AOT.mult, op1=AOT.add)
    out_view = out_d[ch * CH:(ch + 1) * CH].rearrange("(p t) -> p t", p=P)
    nc.sync.dma_start(out_view, ciou)


_CACHE = {}
_ASSIGN_DEFAULT = {"sa": "v", "rb": "g"}


def _get_executable():
    if "exec" in _CACHE:
        return _CACHE["exec"]
    import jax
    from jax.sharding import Mesh, PartitionSpec, NamedSharding
    from jax.experimental.shard_map import shard_map
    from concourse import bass2jax

    nc = build_program(assign=_ASSIGN_DEFAULT)
    bass2jax.install_neuronx_cc_hook()

    partition_name = nc.partition_id_tensor.name if nc.partition_id_tensor else None
    in_names, in_shapes, out_names, out_avals = [], [], [], []
    for alloc in nc.m.functions[0].allocations:
        if not isinstance(alloc, mybir.MemoryLocationSet):
            continue
        name = alloc.memorylocations[0].name
        if alloc.kind == "ExternalInput":
            if name != partition_name:
                in_names.append(name)
                in_shapes.append((tuple(alloc.tensor_shape), mybir.dt.np(alloc.dtype)))
        elif alloc.kind == "ExternalOutput":
            out_names.append(name)
            out_avals.append(jax.core.ShapedArray(
                tuple(alloc.tensor_shape), mybir.dt.np(alloc.dtype)))
    all_names = in_names + out_names
    if partition_name is not None:
        all_names = all_names + [partition_name]

    def _body(*args):
        operands = list(args)
        if partition_name is not None:
            operands.append(bass2jax.partition_id_tensor())
        outs = bass2jax._bass_exec_p.bind(
            *operands,
            out_avals=tuple(out_avals),
            in_names=tuple(all_names),
            out_names=tuple(out_names),
            lowering_input_output_aliases=(),
            sim_require_finite=True,
            sim_require_nnan=True,
            nc=nc,
        )
        return tuple(outs)

    devices = jax.devices()[:NCORES]
    mesh = Mesh(np.asarray(devices), ("core",))
    nin = len(in_names)
    nout = len(out_names)
    sh = NamedSharding(mesh, PartitionSpec("core"))
    jf = shard_map(_body, mesh=mesh,
                   in_specs=(PartitionSpec("core"),) * (nin + nout),
                   out_specs=(PartitionSpec("core"),) * nout,
                   check_rep=False)

    def compile_fn():
        args = [jax.ShapeDtypeStruct((NCORES * s[0], *s[1:]), d, sharding=sh)
                for (s, d) in in_shapes]
        args += [jax.ShapeDtypeStruct((NCORES * av.shape[0], *av.shape[1:]),
                                      av.dtype, sharding=sh)
                 for av in out_avals]
        return jax.jit(jf, keep_unused=True).lower(*args).compile()

    try:
        sharded = bass2jax.fast_dispatch_compile(compile_fn)
    except Exception:
        sharded = jax.jit(jf, keep_unused=True)
    zeros = [np.zeros((NCORES * av.shape[0], *av.shape[1:]), av.dtype)
             for av in out_avals]
    _CACHE["exec"] = (sharded, sh, zeros)
    return _CACHE["exec"]


def kernel(a: np.ndarray, b: np.ndarray) -> np.ndarray:
    import jax
    a8 = np.asarray(a, dtype=np.float32).reshape(NCORES, NI, 16)
    b8 = np.asarray(b, dtype=np.float32).reshape(NCORES, NI, 16)
    ab = np.ascontiguousarray(np.concatenate([a8, b8], axis=2))
    sharded, sh, zeros = _get_executable()
    ab_dev = jax.device_put(ab.reshape(NCORES * NI, 32), sh)
    zeros_dev = [jax.device_put(z, sh) for z in zeros]
    import time as _time
    ciou = None
    for attempt, delay in enumerate((0, 5, 20)):
        if delay:
            _time.sleep(delay)
        try:
            out = sharded(ab_dev, *zeros_dev)
            ciou = np.asarray(out[0], dtype=np.float64)
            break
        except Exception:
            if attempt == 2:
                raise
    return np.float32(ciou.sum() / B)



# revision 2
# speedup vs baseline: 2.5099x; 2.5099x over previous
"""CIoU kernel v4: op-cost-aware redesign from real-HW microbenchmarks.

Key changes vs v3:
  - guarded reciprocal via cody_waite(D, sgD, -eps) + reciprocal_approx_fast
    (replaces Sign/STT/full-reciprocal chain: 24us+7us -> 6.6us per chunk);
  - window trick reworked to clip[0,1] + +-2*sgn shift (B side runs in the
    -SA/D direction: clip[-1,0] + swapped min/max trees);
  - all strided TensorReduces (5.5us each) replaced by slice-halving
    tensor_tensor min/max/add trees (~1.5us each);
  - hull bridge masks via sign algebra: u = sAm - sAf + sBf - sBm in {-4..4},
    mB = u - clip(u, -3, 3) in {-1, 0, +1};
  - Act engine owns all Sign ops, Pool owns t2 + the three clip TSPs;
  - buffers aliased so the whole chunk fits ~198KB/partition SBUF.
"""
import sys

sys.path.insert(0, "/opt/trn_rl_repo")

import numpy as np
import concourse.bass as bass
import concourse.bacc as bacc
import concourse.tile as tile
from concourse import mybir
from concourse.bass_utils import run_bass_kernel_spmd

AOT = mybir.AluOpType
ACT = mybir.ActivationFunctionType
F32 = mybir.dt.float32
F16 = mybir.dt.float16

B = 262144
NCORES = 8
NI = B // NCORES
P = 128
EPS = 1e-6
TINY = 1e-30

ASSIGN = {}


def build_program(T=64, assign=None, npass=1):
    global ASSIGN
    ASSIGN = dict(assign or {})
    CH = P * T
    NCH = NI // CH
    nc = bacc.Bacc("TRN2", target_bir_lowering=False, debug=False, num_devices=NCORES)
    ab_d = nc.dram_tensor("ab", [NI, 32], F32, kind="ExternalInput")
    out_d = nc.dram_tensor("ciou", [NI], F32, kind="ExternalOutput")

    with tile.TileContext(nc) as tc:
        with tc.tile_pool(name="pool", bufs=1) as pool, \
             tc.tile_pool(name="spool", bufs=1) as spool:
            tb = spool.tile([P, 1], F32, tag="tinyb", name="tinyb")
            nc.gpsimd.memset(tb, TINY)
            tm2 = spool.tile([P, 1], F32, tag="tm2b", name="tm2b")
            nc.gpsimd.memset(tm2, -2.0)
            for p_i in range(npass):
                for ch in range(NCH):
                    _chunk(nc, pool, spool, ab_d, out_d, ch, T, tb, tm2)
    nc.compile()
    return nc


def _chunk(nc, pool, spool, ab_d, out_d, ch, T, tb, tm2):
    CH = P * T
    v = nc.vector
    g = nc.gpsimd
    s = nc.scalar

    def E(site, default):
        return {"v": v, "g": g, "s": s}[ASSIGN.get(site, default)]

    def big(tag, d=F32, n=64):
        return pool.tile([P, n * T], d, tag=tag, name=tag)

    def small(tag, d=F32, n=8):
        return spool.tile([P, n * T], d, tag=tag, name=tag)

    def tiny(tag, d=F32):
        return spool.tile([P, T], d, tag=tag, name=tag)

    def dn(tl):
        return tl.rearrange("p (i k t) -> p i k t", i=8, k=8)

    # ---------------- load ----------------
    raw = pool.tile([P, 32 * T], F32, tag="raw", name="raw")
    ab_view = ab_d[ch * CH:(ch + 1) * CH, :].rearrange("(p t) jc -> p (t jc)", p=P)
    nc.sync.dma_start(raw, ab_view)

    rr = raw.rearrange("p (t h j c) -> p h j c t", h=2, j=8, c=2)
    ax = rr[:, 0, :, 0, :]; ay = rr[:, 0, :, 1, :]
    bx = rr[:, 1, :, 0, :]; by = rr[:, 1, :, 1, :]

    # ---------------- adjacency (smalls) ----------------
    va1 = small("va1"); va2 = small("va2")
    v1v = va1.rearrange("p (s t) -> p s t", s=8)
    v2v = va2.rearrange("p (s t) -> p s t", s=8)
    v.tensor_tensor(v1v[:, 0:7, :], ax[:, 0:7, :], ay[:, 1:8, :], AOT.mult)
    v.tensor_tensor(v2v[:, 0:7, :], ay[:, 0:7, :], ax[:, 1:8, :], AOT.mult)
    v.tensor_tensor(v1v[:, 7, :], ax[:, 7, :], ay[:, 0, :], AOT.mult)
    v.tensor_tensor(v2v[:, 7, :], ay[:, 7, :], ax[:, 0, :], AOT.mult)
    adjAB = small("adjab", n=16)
    adjAB_v = adjAB.rearrange("p (s t) -> p s t", s=16)
    adjA_c = adjAB_v[:, 0:8, :]
    v.tensor_tensor(adjA_c, v1v, v2v, AOT.subtract)

    vb1 = small("vb1"); vb2 = small("vb2")
    b1v = vb1.rearrange("p (s t) -> p s t", s=8)
    b2v = vb2.rearrange("p (s t) -> p s t", s=8)
    ge = E("adjb", "v")
    ge.tensor_tensor(b1v[:, 0:7, :], bx[:, 0:7, :], by[:, 1:8, :], AOT.mult)
    ge.tensor_tensor(b2v[:, 0:7, :], by[:, 0:7, :], bx[:, 1:8, :], AOT.mult)
    ge.tensor_tensor(b1v[:, 7, :], bx[:, 7, :], by[:, 0, :], AOT.mult)
    ge.tensor_tensor(b2v[:, 7, :], by[:, 7, :], bx[:, 0, :], AOT.mult)
    adjB_c = adjAB_v[:, 8:16, :]
    ge.tensor_tensor(adjB_c, b1v, b2v, AOT.subtract)

    adjA_ik = adjA_c.unsqueeze(2).broadcast_to((P, 8, 8, T))
    adjB_ik = adjB_c.unsqueeze(1).broadcast_to((P, 8, 8, T))

    # ---------------- C (9x9 padded) ----------------
    A = big("bufA")          # rotating 64T scratch
    Bb = big("bufB")
    ax_b = ax.unsqueeze(2).broadcast_to((P, 8, 8, T))
    ay_b = ay.unsqueeze(2).broadcast_to((P, 8, 8, T))
    bx_b = bx.unsqueeze(1).broadcast_to((P, 8, 8, T))
    by_b = by.unsqueeze(1).broadcast_to((P, 8, 8, T))
    E("t1", "v").tensor_tensor(dn(A), ax_b, by_b, AOT.mult)
    E("t2", "v").tensor_tensor(dn(Bb), ay_b, bx_b, AOT.mult)

    cpad = big("cpad", n=81)
    cp = cpad.rearrange("p (si sk t) -> p si sk t", si=9, sk=9)
    E("c", "v").tensor_tensor(cp[:, 0:8, 0:8, :], dn(A), dn(Bb), AOT.subtract)
    s.copy(cp[:, 0:8, 8, :], cp[:, 0:8, 0, :])      # col 8 <- col 0
    s.copy(cp[:, 8, :, :], cp[:, 0, :, :])          # row 8 <- row 0

    # ---------------- w1 / D / SA / w2 / SB ----------------
    w1p = big("w1p", n=72)
    w1v = w1p.rearrange("p (i q t) -> p i q t", i=8, q=9)
    E("w1", "v").tensor_tensor(w1v, cp[:, 1:9, :, :], cp[:, 0:8, :, :], AOT.subtract)

    D = big("bufD")
    E("d", "v").tensor_tensor(dn(D), w1v[:, :, 1:9, :], w1v[:, :, 0:8, :], AOT.subtract)

    sgD = big("sgd")
    s.sign(sgD, D, bias=tb)                       # Act: sign(D + TINY)

    sap = big("sap", n=72)                          # SA 9 i-slots (SA at 1:9)
    sav = sap.rearrange("p (si k t) -> p si k t", si=9, k=8)
    SA = sav[:, 1:9, :, :]
    E("sa", "v").tensor_tensor(SA, w1v[:, :, 0:8, :], adjA_ik, AOT.add)
    s.copy(sav[:, 0, :, :], sav[:, 8, :, :])

    W2 = A                                          # reuse bufA
    E("w2", "v").tensor_tensor(dn(W2), cp[:, 0:8, 0:8, :], cp[:, 0:8, 1:9, :], AOT.subtract)
    sbp = big("sbp", n=72)                          # SB 9 k-slots (SB at 1:9)
    sbv = sbp.rearrange("p (i sk t) -> p i sk t", i=8, sk=9)
    SB = sbv[:, :, 1:9, :]
    E("sb", "v").tensor_tensor(SB, dn(W2), adjB_ik, AOT.add)
    s.copy(sbv[:, :, 0, :], sbv[:, :, 8, :])

    # ---------------- f16 signs of SA / SB (Act) ----------------
    saf = pool.tile([P, 72 * T], F16, tag="saf", name="saf")
    safv = saf.rearrange("p (si k t) -> p si k t", si=9, k=8)
    s.sign(saf, sap, bias=tb)
    sbf = pool.tile([P, 72 * T], F16, tag="sbf", name="sbf")
    sbfv = sbf.rearrange("p (i sk t) -> p i sk t", i=8, sk=9)
    s.sign(sbf, sbp, bias=tb)

    # ---------------- guarded reciprocal ----------------
    Dsafe = Bb                                      # reuse bufB
    v.cody_waite_cascade(Dsafe, D, sgD, -EPS, 0.0, 0.0)   # D + EPS*sgD
    R = A                                           # bufA dead after SB
    v.reciprocal_approx_fast(out=R, in_=Dsafe)

    # ---------------- A-side windows: ratio=SB/D, rc=clip01, q=rc-2sg ------
    ratioA = Bb                                     # Dsafe dead after R
    E("ra", "v").tensor_tensor(dn(ratioA), SB, dn(R), AOT.mult)
    rcA = D                                         # D dead after Dsafe/sgD
    E("rca", "v").tensor_scalar(rcA, ratioA, 0.0, 1.0, AOT.max, AOT.min)
    qA = w1p[:, 0:64 * T]                           # w1 dead after D/SA
    v.cody_waite_cascade(qA, rcA, sgD, 2.0, 0.0, 0.0)
    qAv = qA.rearrange("p (i k t) -> p i k t", i=8, k=8)

    # trees over k: loA = max_k q, hiA = min_k q
    tr32 = raw                                      # raw dead after t1/t2/adj
    t32a = tr32[:, 0:32 * T].rearrange("p (i k t) -> p i k t", i=8, k=4)
    mid = small("mid", n=16)
    m16 = mid.rearrange("p (i k t) -> p i k t", i=8, k=2)
    loA = small("loa")
    hiA = small("hia")
    v.tensor_tensor(t32a, qAv[:, :, 0:4, :], qAv[:, :, 4:8, :], AOT.max)
    v.tensor_tensor(m16, t32a[:, :, 0:2, :], t32a[:, :, 2:4, :], AOT.max)
    v.tensor_tensor(loA.rearrange("p (i t) -> p i t", i=8),
                    m16[:, :, 0, :], m16[:, :, 1, :], AOT.max)
    v.tensor_tensor(t32a, qAv[:, :, 0:4, :], qAv[:, :, 4:8, :], AOT.min)
    v.tensor_tensor(m16, t32a[:, :, 0:2, :], t32a[:, :, 2:4, :], AOT.min)
    v.tensor_tensor(hiA.rearrange("p (i t) -> p i t", i=8),
                    m16[:, :, 0, :], m16[:, :, 1, :], AOT.min)

    # widths: wA = max(min(hiA+2,1) - max(loA-2,0), 0)
    x1 = va1                                        # small scratch reuse
    x2 = va2
    x3 = small("x3")
    x4 = small("x4")
    s.activation(x1, loA, ACT.Relu, bias=tm2)      # lo2A = relu(loA - 2)
    v.tensor_scalar(x2, hiA, 2.0, 1.0, AOT.add, AOT.min)
    v.tensor_tensor(x3, x2, x1, AOT.subtract)
    iw = small("iw16", n=16)
    iwv = iw.rearrange("p (s t) -> p s t", s=16)
    v.tensor_scalar(x4, x3, 0.0, None, AOT.max)
    v.tensor_tensor(iwv[:, 0:8, :], x4.rearrange("p (i t) -> p i t", i=8),
                    adjA_c, AOT.mult)

    # ---------------- B-side windows (negated direction) ----------------
    ratioB = sbp[:, 0:64 * T]                       # SB dead after ratioA & sbf
    E("rb", "v").tensor_tensor(
        ratioB.rearrange("p (i k t) -> p i k t", i=8, k=8), SA, dn(R), AOT.mult)
    rcBp = D                                        # rcA dead after qA
    E("rcb", "v").tensor_scalar(rcBp, ratioB, -1.0, 0.0, AOT.max, AOT.min)
    qB = Bb                                         # ratioA dead after rcA
    v.cody_waite_cascade(qB, rcBp, sgD, 2.0, 0.0, 0.0)   # qB' = rc' - 2sg  (= -q)
    qBv = qB.rearrange("p (i k t) -> p i k t", i=8, k=8)

    # trees over i; true loB = -min(qB'), hiB = -max(qB')
    t32b = tr32[:, 0:32 * T].rearrange("p (i k t) -> p i k t", i=4, k=8)
    m16b = mid.rearrange("p (i k t) -> p i k t", i=2, k=8)
    mnB = loA                                       # reuse small
    mxB = hiA
    v.tensor_tensor(t32b, qBv[:, 0:4, :, :], qBv[:, 4:8, :, :], AOT.min)
    v.tensor_tensor(m16b, t32b[:, 0:2, :, :], t32b[:, 2:4, :, :], AOT.min)
    v.tensor_tensor(mnB.rearrange("p (k t) -> p k t", k=8),
                    m16b[:, 0, :, :], m16b[:, 1, :, :], AOT.min)
    v.tensor_tensor(t32b, qBv[:, 0:4, :, :], qBv[:, 4:8, :, :], AOT.max)
    v.tensor_tensor(m16b, t32b[:, 0:2, :, :], t32b[:, 2:4, :, :], AOT.max)
    v.tensor_tensor(mxB.rearrange("p (k t) -> p k t", k=8),
                    m16b[:, 0, :, :], m16b[:, 1, :, :], AOT.max)

    # lo2B = relu(-mnB - 2); hB' = max(mxB-2, -1) = -hi2B
    # wB = max(hi2B - lo2B, 0) = max(-(hB' + lo2B), 0)
    s.activation(x1, mnB, ACT.Relu, bias=tm2, scale=-1.0)
    v.tensor_scalar(x2, mxB, -2.0, -1.0, AOT.add, AOT.max)
    v.tensor_tensor(x3, x2, x1, AOT.add)
    v.tensor_scalar(x4, x3, -1.0, 0.0, AOT.mult, AOT.max)
    v.tensor_tensor(iwv[:, 8:16, :], x4.rearrange("p (k t) -> p k t", k=8),
                    adjB_c, AOT.mult)
    isum = tiny("isum")
    v.tensor_reduce(isum, iw.rearrange("p (s t) -> p t s", s=16),
                    axis=mybir.AxisListType.X, op=AOT.add)

    # ---------------- hull: bridges via sign algebra ----------------
    sAm = safv[:, 0:8, :, :]; sAf = safv[:, 1:9, :, :]
    sBm = sbfv[:, :, 0:8, :]; sBf = sbfv[:, :, 1:9, :]
    s1 = R                                          # R dead after ratioB
    E("s1", "v").tensor_tensor(dn(s1), sAm, sBf, AOT.add)
    s2 = sap[:, 0:64 * T]                           # SA dead after ratioB & saf
    E("s2", "v").tensor_tensor(s2.rearrange("p (i k t) -> p i k t", i=8, k=8),
                               sAf, sBm, AOT.add)
    u = sgD                                         # sgD dead after qB
    E("u", "v").tensor_tensor(u, s1, s2, AOT.subtract)
    c2t = D                                         # rcBp dead after qB
    E("c2", "v").tensor_scalar(c2t, u, 3.0, -3.0, AOT.min, AOT.max)
    dd = Bb                                         # qB dead after trees
    E("dd", "v").tensor_tensor(dd, u, c2t, AOT.subtract)
    cM = s1                                         # s1 dead after u
    E("cm", "v").tensor_tensor(dn(cM), dd.rearrange("p (i k t) -> p i k t", i=8, k=8),
                               cp[:, 0:8, 0:8, :], AOT.mult)
    # redBR = sum_{i,k} cM  (tree + small reduce)
    cMv = cM.rearrange("p (i k t) -> p i k t", i=8, k=8)
    v.tensor_tensor(t32a, cMv[:, :, 0:4, :], cMv[:, :, 4:8, :], AOT.add)
    v.tensor_tensor(m16, t32a[:, :, 0:2, :], t32a[:, :, 2:4, :], AOT.add)
    r8 = loA                                        # reuse small
    v.tensor_tensor(r8.rearrange("p (i t) -> p i t", i=8),
                    m16[:, :, 0, :], m16[:, :, 1, :], AOT.add)
    redBR = tiny("redbr")
    v.tensor_reduce(redBR, r8.rearrange("p (s t) -> p t s", s=8),
                    axis=mybir.AxisListType.X, op=AOT.add)

    # ---------------- hull: surviving edges ----------------
    # okEA = all_k sAf >= 0  ->  min-tree over k of sAf (f16 +-1)
    tr16a = tr32[:, 0:16 * T].bitcast(F16).rearrange("p (i k t) -> p i k t", i=8, k=4)
    md16 = mid[:, 0:8 * T].bitcast(F16).rearrange("p (i k t) -> p i k t", i=8, k=2)
    mnsA = spool.tile([P, 8 * T], F16, tag="mns", name="mns")
    v.tensor_tensor(tr16a, sAf[:, :, 0:4, :], sAf[:, :, 4:8, :], AOT.min)
    v.tensor_tensor(md16, tr16a[:, :, 0:2, :], tr16a[:, :, 2:4, :], AOT.min)
    v.tensor_tensor(mnsA.rearrange("p (i t) -> p i t", i=8),
                    md16[:, :, 0, :], md16[:, :, 1, :], AOT.min)
    v.tensor_scalar(x1, mnsA, 1.0, 0.5, AOT.add, AOT.mult)     # okEA in {0,1}
    hw = iw                                         # reuse iw16 after isum
    hwv = hw.rearrange("p (s t) -> p s t", s=16)
    v.tensor_tensor(hwv[:, 0:8, :], x1.rearrange("p (i t) -> p i t", i=8),
                    adjA_c, AOT.mult)
    tr16b = tr32[:, 0:16 * T].bitcast(F16).rearrange("p (i k t) -> p i k t", i=4, k=8)
    md16b = mid[:, 0:8 * T].bitcast(F16).rearrange("p (i k t) -> p i k t", i=2, k=8)
    v.tensor_tensor(tr16b, sBf[:, 0:4, :, :], sBf[:, 4:8, :, :], AOT.min)
    v.tensor_tensor(md16b, tr16b[:, 0:2, :, :], tr16b[:, 2:4, :, :], AOT.min)
    v.tensor_tensor(mnsA.rearrange("p (k t) -> p k t", k=8),
                    md16b[:, 0, :, :], md16b[:, 1, :, :], AOT.min)
    v.tensor_scalar(x1, mnsA, 1.0, 0.5, AOT.add, AOT.mult)
    v.tensor_tensor(hwv[:, 8:16, :], x1.rearrange("p (k t) -> p k t", k=8),
                    adjB_c, AOT.mult)
    h1 = tiny("h1")
    v.tensor_reduce(h1, hw.rearrange("p (s t) -> p t s", s=16),
                    axis=mybir.AxisListType.X, op=AOT.add)

    # ---------------- per-item finals ----------------
    asum = tiny("asum")
    v.tensor_reduce(asum, adjAB.rearrange("p (s t) -> p t s", s=16),
                    axis=mybir.AxisListType.X, op=AOT.add)
    hsum = tiny("hsum")
    v.tensor_tensor(hsum, h1, redBR, AOT.add)       # = 2*hull_area
    u2 = tiny("u2")
    v.tensor_tensor(u2, asum, isum, AOT.subtract)   # = 2*union
    ru2 = tiny("ru2")
    v.reciprocal_approx_fast(out=ru2, in_=u2)
    rh = tiny("rh")
    v.reciprocal_approx_fast(out=rh, in_=hsum)
    iou = tiny("iou")
    v.tensor_tensor(iou, isum, ru2, AOT.mult)
    t3 = tiny("t3")
    v.tensor_tensor(t3, u2, rh, AOT.mult)
    ciou = tiny("ciou")
    v.affine_then_add(ciou, iou, t3, scale=1.0, bias=-1.0)
    out_view = out_d[ch * CH:(ch + 1) * CH].rearrange("(p t) -> p t", p=P)
    nc.sync.dma_start(out_view, ciou)


_CACHE = {}
_ASSIGN_DEFAULT = {}


def _get_executable(npass=1):
    key = ("exec", npass)
    if key in _CACHE:
        return _CACHE[key]
    import jax
    from jax.sharding import Mesh, PartitionSpec, NamedSharding
    from jax.experimental.shard_map import shard_map
    from concourse import bass2jax

    nc = build_program(assign=_ASSIGN_DEFAULT, npass=npass)
    bass2jax.install_neuronx_cc_hook()

    partition_name = nc.partition_id_tensor.name if nc.partition_id_tensor else None
    in_names, in_shapes, out_names, out_avals = [], [], [], []
    for alloc in nc.m.functions[0].allocations:
        if not isinstance(alloc, mybir.MemoryLocationSet):
            continue
        name = alloc.memorylocations[0].name
        if alloc.kind == "ExternalInput":
            if name != partition_name:
                in_names.append(name)
                in_shapes.append((tuple(alloc.tensor_shape), mybir.dt.np(alloc.dtype)))
        elif alloc.kind == "ExternalOutput":
            out_names.append(name)
            out_avals.append(jax.core.ShapedArray(
                tuple(alloc.tensor_shape), mybir.dt.np(alloc.dtype)))
    all_names = in_names + out_names
    if partition_name is not None:
        all_names = all_names + [partition_name]

    def _body(*args):
        operands = list(args)
        if partition_name is not None:
            operands.append(bass2jax.partition_id_tensor())
        outs = bass2jax._bass_exec_p.bind(
            *operands,
            out_avals=tuple(out_avals),
            in_names=tuple(all_names),
            out_names=tuple(out_names),
            lowering_input_output_aliases=(),
            sim_require_finite=True,
            sim_require_nnan=True,
            nc=nc,
        )
        return tuple(outs)

    devices = jax.devices()[:NCORES]
    mesh = Mesh(np.asarray(devices), ("core",))
    nin = len(in_names)
    nout = len(out_names)
    sh = NamedSharding(mesh, PartitionSpec("core"))
    jf = shard_map(_body, mesh=mesh,
                   in_specs=(PartitionSpec("core"),) * (nin + nout),
                   out_specs=(PartitionSpec("core"),) * nout,
                   check_rep=False)

    def compile_fn():
        args = [jax.ShapeDtypeStruct((NCORES * s[0], *s[1:]), d, sharding=sh)
                for (s, d) in in_shapes]
        args += [jax.ShapeDtypeStruct((NCORES * av.shape[0], *av.shape[1:]),
                                      av.dtype, sharding=sh)
                 for av in out_avals]
        return jax.jit(jf, keep_unused=True).lower(*args).compile()

    try:
        sharded = bass2jax.fast_dispatch_compile(compile_fn)
    except Exception:
        sharded = jax.jit(jf, keep_unused=True)
    zeros = [np.zeros((NCORES * av.shape[0], *av.shape[1:]), av.dtype)
             for av in out_avals]
    _CACHE[key] = (sharded, sh, zeros)
    return _CACHE[key]


def kernel(a: np.ndarray, b: np.ndarray) -> np.ndarray:
    import jax
    a8 = np.asarray(a, dtype=np.float32).reshape(NCORES, NI, 16)
    b8 = np.asarray(b, dtype=np.float32).reshape(NCORES, NI, 16)
    ab = np.ascontiguousarray(np.concatenate([a8, b8], axis=2))
    sharded, sh, zeros = _get_executable()
    ab_dev = jax.device_put(ab.reshape(NCORES * NI, 32), sh)
    zeros_dev = [jax.device_put(z, sh) for z in zeros]
    import time as _time
    ciou = None
    for attempt, delay in enumerate((0, 5, 20)):
        if delay:
            _time.sleep(delay)
        try:
            out = sharded(ab_dev, *zeros_dev)
            ciou = np.asarray(out[0], dtype=np.float64)
            break
        except Exception:
            if attempt == 2:
                raise
    return np.float32(ciou.sum() / B)


# revision 6
# speedup vs baseline: 2.5114x; 1.0006x over previous
"""CIoU kernel v4: op-cost-aware redesign from real-HW microbenchmarks.

Key changes vs v3:
  - guarded reciprocal via cody_waite(D, sgD, -eps) + reciprocal_approx_fast
    (replaces Sign/STT/full-reciprocal chain: 24us+7us -> 6.6us per chunk);
  - window trick reworked to clip[0,1] + +-2*sgn shift (B side runs in the
    -SA/D direction: clip[-1,0] + swapped min/max trees);
  - all strided TensorReduces (5.5us each) replaced by slice-halving
    tensor_tensor min/max/add trees (~1.5us each);
  - hull bridge masks via sign algebra: u = sAm - sAf + sBf - sBm in {-4..4},
    mB = u - clip(u, -3, 3) in {-1, 0, +1};
  - Act engine owns all Sign ops, Pool owns t2 + the three clip TSPs;
  - buffers aliased so the whole chunk fits ~198KB/partition SBUF.
"""
import sys

sys.path.insert(0, "/opt/trn_rl_repo")

import numpy as np
import concourse.bass as bass
import concourse.bacc as bacc
import concourse.tile as tile
from concourse import mybir
from concourse.bass_utils import run_bass_kernel_spmd

AOT = mybir.AluOpType
ACT = mybir.ActivationFunctionType
F32 = mybir.dt.float32
F16 = mybir.dt.float16

B = 262144
NCORES = 8
NI = B // NCORES
P = 128
EPS = 1e-6
TINY = 1e-30

ASSIGN = {}


def build_program(T=64, assign=None, npass=1):
    global ASSIGN
    ASSIGN = dict(assign or {})
    CH = P * T
    NCH = NI // CH
    nc = bacc.Bacc("TRN2", target_bir_lowering=False, debug=False, num_devices=NCORES)
    ab_d = nc.dram_tensor("ab", [NI, 32], F32, kind="ExternalInput")
    out_d = nc.dram_tensor("ciou", [NI], F32, kind="ExternalOutput")

    with tile.TileContext(nc) as tc:
        with tc.tile_pool(name="pool", bufs=1) as pool, \
             tc.tile_pool(name="spool", bufs=1) as spool:
            tb = spool.tile([P, 1], F32, tag="tinyb", name="tinyb")
            nc.gpsimd.memset(tb, TINY)
            tm2 = spool.tile([P, 1], F32, tag="tm2b", name="tm2b")
            nc.gpsimd.memset(tm2, -2.0)
            for p_i in range(npass):
                for ch in range(NCH):
                    _chunk(nc, pool, spool, ab_d, out_d, ch, T, tb, tm2)
    nc.compile()
    return nc


def _chunk(nc, pool, spool, ab_d, out_d, ch, T, tb, tm2):
    CH = P * T
    v = nc.vector
    g = nc.gpsimd
    s = nc.scalar

    def E(site, default):
        return {"v": v, "g": g, "s": s}[ASSIGN.get(site, default)]

    def big(tag, d=F32, n=64):
        return pool.tile([P, n * T], d, tag=tag, name=tag)

    def small(tag, d=F32, n=8):
        return spool.tile([P, n * T], d, tag=tag, name=tag)

    def tiny(tag, d=F32):
        return spool.tile([P, T], d, tag=tag, name=tag)

    def dn(tl):
        return tl.rearrange("p (i k t) -> p i k t", i=8, k=8)

    # ---------------- load ----------------
    raw = pool.tile([P, 32 * T], F32, tag="raw", name="raw")
    ab_view = ab_d[ch * CH:(ch + 1) * CH, :].rearrange("(p t) jc -> p (t jc)", p=P)
    nc.sync.dma_start(raw, ab_view)

    rr = raw.rearrange("p (t h j c) -> p h j c t", h=2, j=8, c=2)
    ax = rr[:, 0, :, 0, :]; ay = rr[:, 0, :, 1, :]
    bx = rr[:, 1, :, 0, :]; by = rr[:, 1, :, 1, :]

    # ---------------- adjacency (smalls) ----------------
    va1 = small("va1"); va2 = small("va2")
    v1v = va1.rearrange("p (s t) -> p s t", s=8)
    v2v = va2.rearrange("p (s t) -> p s t", s=8)
    v.tensor_tensor(v1v[:, 0:7, :], ax[:, 0:7, :], ay[:, 1:8, :], AOT.mult)
    v.tensor_tensor(v2v[:, 0:7, :], ay[:, 0:7, :], ax[:, 1:8, :], AOT.mult)
    v.tensor_tensor(v1v[:, 7, :], ax[:, 7, :], ay[:, 0, :], AOT.mult)
    v.tensor_tensor(v2v[:, 7, :], ay[:, 7, :], ax[:, 0, :], AOT.mult)
    adjAB = small("adjab", n=16)
    adjAB_v = adjAB.rearrange("p (s t) -> p s t", s=16)
    adjA_c = adjAB_v[:, 0:8, :]
    v.tensor_tensor(adjA_c, v1v, v2v, AOT.subtract)

    vb1 = small("vb1"); vb2 = small("vb2")
    b1v = vb1.rearrange("p (s t) -> p s t", s=8)
    b2v = vb2.rearrange("p (s t) -> p s t", s=8)
    ge = E("adjb", "v")
    ge.tensor_tensor(b1v[:, 0:7, :], bx[:, 0:7, :], by[:, 1:8, :], AOT.mult)
    ge.tensor_tensor(b2v[:, 0:7, :], by[:, 0:7, :], bx[:, 1:8, :], AOT.mult)
    ge.tensor_tensor(b1v[:, 7, :], bx[:, 7, :], by[:, 0, :], AOT.mult)
    ge.tensor_tensor(b2v[:, 7, :], by[:, 7, :], bx[:, 0, :], AOT.mult)
    adjB_c = adjAB_v[:, 8:16, :]
    ge.tensor_tensor(adjB_c, b1v, b2v, AOT.subtract)

    adjA_ik = adjA_c.unsqueeze(2).broadcast_to((P, 8, 8, T))
    adjB_ik = adjB_c.unsqueeze(1).broadcast_to((P, 8, 8, T))

    # ---------------- C (9x9 padded) ----------------
    A = big("bufA")          # rotating 64T scratch
    Bb = big("bufB")
    ax_b = ax.unsqueeze(2).broadcast_to((P, 8, 8, T))
    ay_b = ay.unsqueeze(2).broadcast_to((P, 8, 8, T))
    bx_b = bx.unsqueeze(1).broadcast_to((P, 8, 8, T))
    by_b = by.unsqueeze(1).broadcast_to((P, 8, 8, T))
    E("t1", "v").tensor_tensor(dn(A), ax_b, by_b, AOT.mult)
    E("t2", "v").tensor_tensor(dn(Bb), ay_b, bx_b, AOT.mult)

    cpad = big("cpad", n=81)
    cp = cpad.rearrange("p (si sk t) -> p si sk t", si=9, sk=9)
    E("c", "v").tensor_tensor(cp[:, 0:8, 0:8, :], dn(A), dn(Bb), AOT.subtract)
    s.copy(cp[:, 0:8, 8, :], cp[:, 0:8, 0, :])      # col 8 <- col 0
    s.copy(cp[:, 8, :, :], cp[:, 0, :, :])          # row 8 <- row 0

    # ---------------- w1 / D / SA / w2 / SB ----------------
    w1p = big("w1p", n=72)
    w1v = w1p.rearrange("p (i q t) -> p i q t", i=8, q=9)
    E("w1", "v").tensor_tensor(w1v, cp[:, 1:9, :, :], cp[:, 0:8, :, :], AOT.subtract)

    D = big("bufD")
    E("d", "v").tensor_tensor(dn(D), w1v[:, :, 1:9, :], w1v[:, :, 0:8, :], AOT.subtract)

    sgD = big("sgd")
    s.sign(sgD, D, bias=tb)                       # Act: sign(D + TINY)

    sap = big("sap", n=72)                          # SA 9 i-slots (SA at 1:9)
    sav = sap.rearrange("p (si k t) -> p si k t", si=9, k=8)
    SA = sav[:, 1:9, :, :]
    E("sa", "v").tensor_tensor(SA, w1v[:, :, 0:8, :], adjA_ik, AOT.add)
    s.copy(sav[:, 0, :, :], sav[:, 8, :, :])

    W2 = A                                          # reuse bufA
    E("w2", "v").tensor_tensor(dn(W2), cp[:, 0:8, 0:8, :], cp[:, 0:8, 1:9, :], AOT.subtract)
    sbp = big("sbp", n=72)                          # SB 9 k-slots (SB at 1:9)
    sbv = sbp.rearrange("p (i sk t) -> p i sk t", i=8, sk=9)
    SB = sbv[:, :, 1:9, :]
    E("sb", "v").tensor_tensor(SB, dn(W2), adjB_ik, AOT.add)
    s.copy(sbv[:, :, 0, :], sbv[:, :, 8, :])

    # ---------------- f16 signs of SA / SB (Act) ----------------
    saf = pool.tile([P, 72 * T], F16, tag="saf", name="saf")
    safv = saf.rearrange("p (si k t) -> p si k t", si=9, k=8)
    s.sign(saf, sap, bias=tb)
    sbf = pool.tile([P, 72 * T], F16, tag="sbf", name="sbf")
    sbfv = sbf.rearrange("p (i sk t) -> p i sk t", i=8, sk=9)
    s.sign(sbf, sbp, bias=tb)

    # ---------------- guarded reciprocal ----------------
    Dsafe = Bb                                      # reuse bufB
    v.cody_waite_cascade(Dsafe, D, sgD, -EPS, 0.0, 0.0)   # D + EPS*sgD
    R = A                                           # bufA dead after SB
    v.reciprocal_approx_fast(out=R, in_=Dsafe)

    # ---------------- A-side windows: ratio=SB/D, rc=clip01, q=rc-2sg ------
    ratioA = Bb                                     # Dsafe dead after R
    E("ra", "v").tensor_tensor(dn(ratioA), SB, dn(R), AOT.mult)
    rcA = D                                         # D dead after Dsafe/sgD
    E("rca", "v").tensor_scalar(rcA, ratioA, 0.0, 1.0, AOT.max, AOT.min)
    qA = w1p[:, 0:64 * T]                           # w1 dead after D/SA
    v.cody_waite_cascade(qA, rcA, sgD, 2.0, 0.0, 0.0)
    qAv = qA.rearrange("p (i k t) -> p i k t", i=8, k=8)

    # trees over k into unified 16-slot lo/hi (A: slots 0:8, B: 8:16)
    tr32 = raw                                      # raw dead after t1/t2/adj
    t32a = tr32[:, 0:32 * T].rearrange("p (i k t) -> p i k t", i=8, k=4)
    mid = small("mid", n=16)
    m16 = mid.rearrange("p (i k t) -> p i k t", i=8, k=2)
    lo16 = small("lo16", n=16)
    hi16 = small("hi16", n=16)
    lo16v = lo16.rearrange("p (s t) -> p s t", s=16)
    hi16v = hi16.rearrange("p (s t) -> p s t", s=16)
    v.tensor_tensor(t32a, qAv[:, :, 0:4, :], qAv[:, :, 4:8, :], AOT.max)
    v.tensor_tensor(m16, t32a[:, :, 0:2, :], t32a[:, :, 2:4, :], AOT.max)
    v.tensor_tensor(lo16v[:, 0:8, :], m16[:, :, 0, :], m16[:, :, 1, :], AOT.max)
    v.tensor_tensor(t32a, qAv[:, :, 0:4, :], qAv[:, :, 4:8, :], AOT.min)
    v.tensor_tensor(m16, t32a[:, :, 0:2, :], t32a[:, :, 2:4, :], AOT.min)
    v.tensor_tensor(hi16v[:, 0:8, :], m16[:, :, 0, :], m16[:, :, 1, :], AOT.min)

    # ---------------- B-side windows (negated at write: q = -(rc'-2sg)) ----
    ratioB = sbp[:, 0:64 * T]                       # SB dead after ratioA & sbf
    E("rb", "v").tensor_tensor(
        ratioB.rearrange("p (i k t) -> p i k t", i=8, k=8), SA, dn(R), AOT.mult)
    rcBp = D                                        # rcA dead after qA
    E("rcb", "v").tensor_scalar(rcBp, ratioB, -1.0, 0.0, AOT.max, AOT.min)
    qB = Bb                                         # ratioA dead after rcA
    v.ln_bwd_dx(qB, rcBp, sgD, 2.0, 0.0, -1.0)      # qB = -(rc' - 2sg)
    qBv = qB.rearrange("p (i k t) -> p i k t", i=8, k=8)

    # trees over i (same polarity as A side now)
    t32b = tr32[:, 0:32 * T].rearrange("p (i k t) -> p i k t", i=4, k=8)
    m16b = mid.rearrange("p (i k t) -> p i k t", i=2, k=8)
    v.tensor_tensor(t32b, qBv[:, 0:4, :, :], qBv[:, 4:8, :, :], AOT.max)
    v.tensor_tensor(m16b, t32b[:, 0:2, :, :], t32b[:, 2:4, :, :], AOT.max)
    v.tensor_tensor(lo16v[:, 8:16, :], m16b[:, 0, :, :], m16b[:, 1, :, :], AOT.max)
    v.tensor_tensor(t32b, qBv[:, 0:4, :, :], qBv[:, 4:8, :, :], AOT.min)
    v.tensor_tensor(m16b, t32b[:, 0:2, :, :], t32b[:, 2:4, :, :], AOT.min)
    v.tensor_tensor(hi16v[:, 8:16, :], m16b[:, 0, :, :], m16b[:, 1, :, :], AOT.min)

    # unified widths: w = max(min(hi+2,1) - max(lo-2,0), 0); iw = w*adj
    x1 = small("x1", n=16)
    x2 = small("x2", n=16)
    iw = small("iw16", n=16)
    x3 = iw                    # WAR-safe: iw's own write reads only x4/adjAB
    x4 = small("x4", n=16)
    s.activation(x1, lo16, ACT.Relu, bias=tm2)      # relu(lo - 2)
    v.tensor_scalar(x2, hi16, 2.0, 1.0, AOT.add, AOT.min)
    v.tensor_tensor(x3, x2, x1, AOT.subtract)
    v.tensor_scalar(x4, x3, 0.0, None, AOT.max)
    iwv = iw.rearrange("p (s t) -> p s t", s=16)
    v.tensor_tensor(iw, x4, adjAB, AOT.mult)
    isum = tiny("isum")
    v.tensor_reduce(isum, iw.rearrange("p (s t) -> p t s", s=16),
                    axis=mybir.AxisListType.X, op=AOT.add)

    # ---------------- hull: bridges via sign algebra ----------------
    sAm = safv[:, 0:8, :, :]; sAf = safv[:, 1:9, :, :]
    sBm = sbfv[:, :, 0:8, :]; sBf = sbfv[:, :, 1:9, :]
    s1 = R                                          # R dead after ratioB
    E("s1", "v").tensor_tensor(dn(s1), sAm, sBf, AOT.add)
    s2 = sap[:, 0:64 * T]                           # SA dead after ratioB & saf
    E("s2", "v").tensor_tensor(s2.rearrange("p (i k t) -> p i k t", i=8, k=8),
                               sAf, sBm, AOT.add)
    u = sgD                                         # sgD dead after qB
    E("u", "v").tensor_tensor(u, s1, s2, AOT.subtract)
    c2t = D                                         # rcBp dead after qB
    E("c2", "v").tensor_scalar(c2t, u, 3.0, -3.0, AOT.min, AOT.max)
    dd = Bb                                         # qB dead after trees
    E("dd", "v").tensor_tensor(dd, u, c2t, AOT.subtract)
    cM = s1                                         # s1 dead after u
    E("cm", "v").tensor_tensor(dn(cM), dd.rearrange("p (i k t) -> p i k t", i=8, k=8),
                               cp[:, 0:8, 0:8, :], AOT.mult)
    # redBR = sum_{i,k} cM  (tree + small reduce)
    cMv = cM.rearrange("p (i k t) -> p i k t", i=8, k=8)
    v.tensor_tensor(t32a, cMv[:, :, 0:4, :], cMv[:, :, 4:8, :], AOT.add)
    v.tensor_tensor(m16, t32a[:, :, 0:2, :], t32a[:, :, 2:4, :], AOT.add)
    r8 = lo16[:, 0:8 * T]                           # reuse small
    v.tensor_tensor(r8.rearrange("p (i t) -> p i t", i=8),
                    m16[:, :, 0, :], m16[:, :, 1, :], AOT.add)
    redBR = tiny("redbr")
    v.tensor_reduce(redBR, r8.rearrange("p (s t) -> p t s", s=8),
                    axis=mybir.AxisListType.X, op=AOT.add)

    # ---------------- hull: surviving edges ----------------
    # okEA = all_k sAf >= 0  ->  min-tree over k of sAf (f16 +-1)
    tr16a = tr32[:, 0:16 * T].bitcast(F16).rearrange("p (i k t) -> p i k t", i=8, k=4)
    md16 = mid[:, 0:8 * T].bitcast(F16).rearrange("p (i k t) -> p i k t", i=8, k=2)
    mnsA = va1.bitcast(F16)[:, 0:8 * T]             # va1 dead after adjacency
    v.tensor_tensor(tr16a, sAf[:, :, 0:4, :], sAf[:, :, 4:8, :], AOT.min)
    v.tensor_tensor(md16, tr16a[:, :, 0:2, :], tr16a[:, :, 2:4, :], AOT.min)
    v.tensor_tensor(mnsA.rearrange("p (i t) -> p i t", i=8),
                    md16[:, :, 0, :], md16[:, :, 1, :], AOT.min)
    okA = hi16[:, 0:8 * T]                          # hi16 dead after widths
    v.tensor_scalar(okA, mnsA, 1.0, 0.5, AOT.add, AOT.mult)    # okEA in {0,1}
    hw = iw                                         # reuse iw16 after isum
    hwv = hw.rearrange("p (s t) -> p s t", s=16)
    v.tensor_tensor(hwv[:, 0:8, :], okA.rearrange("p (i t) -> p i t", i=8),
                    adjA_c, AOT.mult)
    tr16b = tr32[:, 0:16 * T].bitcast(F16).rearrange("p (i k t) -> p i k t", i=4, k=8)
    md16b = mid[:, 0:8 * T].bitcast(F16).rearrange("p (i k t) -> p i k t", i=2, k=8)
    v.tensor_tensor(tr16b, sBf[:, 0:4, :, :], sBf[:, 4:8, :, :], AOT.min)
    v.tensor_tensor(md16b, tr16b[:, 0:2, :, :], tr16b[:, 2:4, :, :], AOT.min)
    v.tensor_tensor(mnsA.rearrange("p (k t) -> p k t", k=8),
                    md16b[:, 0, :, :], md16b[:, 1, :, :], AOT.min)
    okB = hi16[:, 8 * T:16 * T]
    v.tensor_scalar(okB, mnsA, 1.0, 0.5, AOT.add, AOT.mult)
    v.tensor_tensor(hwv[:, 8:16, :], okB.rearrange("p (k t) -> p k t", k=8),
                    adjB_c, AOT.mult)
    h1 = tiny("h1")
    v.tensor_reduce(h1, hw.rearrange("p (s t) -> p t s", s=16),
                    axis=mybir.AxisListType.X, op=AOT.add)

    # ---------------- per-item finals ----------------
    asum = tiny("asum")
    v.tensor_reduce(asum, adjAB.rearrange("p (s t) -> p t s", s=16),
                    axis=mybir.AxisListType.X, op=AOT.add)
    hsum = tiny("hsum")
    v.tensor_tensor(hsum, h1, redBR, AOT.add)       # = 2*hull_area
    u2 = tiny("u2")
    v.tensor_tensor(u2, asum, isum, AOT.subtract)   # = 2*union
    ru2 = tiny("ru2")
    v.reciprocal_approx_fast(out=ru2, in_=u2)
    rh = tiny("rh")
    v.reciprocal_approx_fast(out=rh, in_=hsum)
    iou = tiny("iou")
    v.tensor_tensor(iou, isum, ru2, AOT.mult)
    t3 = tiny("t3")
    v.tensor_tensor(t3, u2, rh, AOT.mult)
    ciou = tiny("ciou")
    v.affine_then_add(ciou, iou, t3, scale=1.0, bias=-1.0)
    out_view = out_d[ch * CH:(ch + 1) * CH].rearrange("(p t) -> p t", p=P)
    nc.sync.dma_start(out_view, ciou)


_CACHE = {}
_ASSIGN_DEFAULT = {}


def _get_executable(npass=1):
    key = ("exec", npass)
    if key in _CACHE:
        return _CACHE[key]
    import jax
    from jax.sharding import Mesh, PartitionSpec, NamedSharding
    from jax.experimental.shard_map import shard_map
    from concourse import bass2jax

    nc = build_program(assign=_ASSIGN_DEFAULT, npass=npass)
    bass2jax.install_neuronx_cc_hook()

    partition_name = nc.partition_id_tensor.name if nc.partition_id_tensor else None
    in_names, in_shapes, out_names, out_avals = [], [], [], []
    for alloc in nc.m.functions[0].allocations:
        if not isinstance(alloc, mybir.MemoryLocationSet):
            continue
        name = alloc.memorylocations[0].name
        if alloc.kind == "ExternalInput":
            if name != partition_name:
                in_names.append(name)
                in_shapes.append((tuple(alloc.tensor_shape), mybir.dt.np(alloc.dtype)))
        elif alloc.kind == "ExternalOutput":
            out_names.append(name)
            out_avals.append(jax.core.ShapedArray(
                tuple(alloc.tensor_shape), mybir.dt.np(alloc.dtype)))
    all_names = in_names + out_names
    if partition_name is not None:
        all_names = all_names + [partition_name]

    def _body(*args):
        operands = list(args)
        if partition_name is not None:
            operands.append(bass2jax.partition_id_tensor())
        outs = bass2jax._bass_exec_p.bind(
            *operands,
            out_avals=tuple(out_avals),
            in_names=tuple(all_names),
            out_names=tuple(out_names),
            lowering_input_output_aliases=(),
            sim_require_finite=True,
            sim_require_nnan=True,
            nc=nc,
        )
        return tuple(outs)

    devices = jax.devices()[:NCORES]
    mesh = Mesh(np.asarray(devices), ("core",))
    nin = len(in_names)
    nout = len(out_names)
    sh = NamedSharding(mesh, PartitionSpec("core"))
    jf = shard_map(_body, mesh=mesh,
                   in_specs=(PartitionSpec("core"),) * (nin + nout),
                   out_specs=(PartitionSpec("core"),) * nout,
                   check_rep=False)

    def compile_fn():
        args = [jax.ShapeDtypeStruct((NCORES * s[0], *s[1:]), d, sharding=sh)
                for (s, d) in in_shapes]
        args += [jax.ShapeDtypeStruct((NCORES * av.shape[0], *av.shape[1:]),
                                      av.dtype, sharding=sh)
                 for av in out_avals]
        return jax.jit(jf, keep_unused=True).lower(*args).compile()

    try:
        sharded = bass2jax.fast_dispatch_compile(compile_fn)
    except Exception:
        sharded = jax.jit(jf, keep_unused=True)
    zeros = [np.zeros((NCORES * av.shape[0], *av.shape[1:]), av.dtype)
             for av in out_avals]
    _CACHE[key] = (sharded, sh, zeros)
    return _CACHE[key]


def kernel(a: np.ndarray, b: np.ndarray) -> np.ndarray:
    import jax
    a8 = np.asarray(a, dtype=np.float32).reshape(NCORES, NI, 16)
    b8 = np.asarray(b, dtype=np.float32).reshape(NCORES, NI, 16)
    ab = np.ascontiguousarray(np.concatenate([a8, b8], axis=2))
    sharded, sh, zeros = _get_executable()
    ab_dev = jax.device_put(ab.reshape(NCORES * NI, 32), sh)
    zeros_dev = [jax.device_put(z, sh) for z in zeros]
    import time as _time
    ciou = None
    for attempt, delay in enumerate((0, 5, 20)):
        if delay:
            _time.sleep(delay)
        try:
            out = sharded(ab_dev, *zeros_dev)
            ciou = np.asarray(out[0], dtype=np.float64)
            break
        except Exception:
            if attempt == 2:
                raise
    return np.float32(ciou.sum() / B)
